# revision 9
# baseline (speedup 1.0000x reference)
"""Trainium2 Bass kernel for the CNF reversible backward solve.

Math restructuring (exact, validated in fp64 against the jax reference):

The per-step recursion
    f1 = W2 tanh(W1 y + b(t1)) + b2
    z' = z - h f1
    f0 = W2 tanh(W1 z' + b(t0)) + b2
    y' = inv_l y + (1-inv_l) z' - inv_l h f0
is tracked purely in H-space (H=256) via Z = W1 z, Y = W1 y:
    a_even = tanh(Y + beta_even)        [scalar engine, per-partition bias]
    Z     += Mz @ a_even                [PSUM-resident, Mz = -h W1 W2]
    a_odd  = tanh(Z + beta_odd)
    Y'     = inv_l Y + (1-inv_l) Z + inv_l (Mz @ a_odd)  [fused DVE ops]

The device computes ONLY this chain and streams all activations a_e out to
DRAM. The D-space outputs are exact fp64 host-side postprocessing:
    y_final = c_y y1 + sum_e gamma_e (W2 @ a_e) + c_b b2
    I_final = h (N * sum(c) - sum_s c . a_even_s^2),  c = diag(W1 W2)

Sharding: data-parallel, B=256 -> 32 samples on each of 8 cores; all
parameters/tables replicated. Output gathered and assembled on host.
"""

import numpy as np
import ml_dtypes
from contextlib import ExitStack

import concourse.bass as bass
import concourse.tile as tile
from concourse import bacc, mybir
from concourse.bass_utils import run_bass_kernel_spmd

# Problem constants (hardcoded per contract)
NCORES = 8
B, D, H = 256, 64, 256
NSTEP = 64
HSTEP = 1.0 / NSTEP
LCOUP = 0.999
INVL = 1.0 / LCOUP
BS = B // NCORES  # 32 samples per core
NBLK = H // 128  # 2 h-blocks
FREE = NBLK * BS  # 64: free size of H-space tiles, layout (blk, sample)
NEVAL = 2 * NSTEP  # 128
ACOLS = NSTEP * FREE  # 4096 columns in each activation stream buffer
DMA_CHUNKS = 8
CSTEPS = NSTEP // DMA_CHUNKS  # steps per out-DMA chunk

F32 = mybir.dt.float32
BF16 = mybir.dt.bfloat16
BF16NP = ml_dtypes.bfloat16


def _coefficients():
    """Exact fp64 scalar recursions for the output-extraction weights."""
    gamma = np.zeros(NEVAL)
    la = np.zeros(NEVAL)
    alpha_y = alpha_z = 1.0
    nu_y = nu_z = 0.0
    for s in range(NSTEP):
        la[2 * s] += -HSTEP
        nu_z += -HSTEP
        gamma *= INVL
        alpha_y *= INVL
        nu_y *= INVL
        gamma += (1.0 - INVL) * la
        alpha_y += (1.0 - INVL) * alpha_z
        nu_y += (1.0 - INVL) * nu_z
        gamma[2 * s + 1] += -INVL * HSTEP
        nu_y += -INVL * HSTEP
    return gamma, alpha_y, nu_y


def _host_tables(W1, b1, u1, W2, b2):
    """All precomputed tensors, fp64 internally."""
    W1 = W1.astype(np.float64)
    W2 = W2.astype(np.float64)
    b1 = b1.astype(np.float64)
    u1 = u1.astype(np.float64)
    b2 = b2.astype(np.float64)

    Mz = -HSTEP * (W1 @ W2)  # [H, H]
    W1b2 = W1 @ b2  # [H]

    kappa = INVL ** (-np.arange(NSTEP + 1, dtype=np.float64))
    d = np.zeros((NSTEP + 1, H))
    for s in range(NSTEP):
        d[s + 1] = d[s] - kappa[s] * HSTEP * W1b2

    # mzt_pack[p, (k*NBLK+j)*128 + m] = Mz[128*j+m, 128*k+p]
    MzT = Mz.T  # [K(h), M(h')]
    mzt_pack = np.zeros((128, NBLK * NBLK * 128))
    for k in range(NBLK):
        for j in range(NBLK):
            mzt_pack[:, (k * NBLK + j) * 128 : (k * NBLK + j + 1) * 128] = MzT[
                128 * k : 128 * k + 128, 128 * j : 128 * j + 128
            ]

    # bias tables [128, NSTEP*NBLK], col s*NBLK+blk
    be_pack = np.zeros((128, NSTEP * NBLK))
    bo_pack = np.zeros((128, NSTEP * NBLK))
    for s in range(NSTEP):
        t1 = 1.0 - s * HSTEP
        t0 = 1.0 - (s + 1) * HSTEP
        bias_even = b1 + t1 * u1 - (1.0 / kappa[s]) * d[s]
        bias_odd = b1 + t0 * u1 - (s + 1) * HSTEP * W1b2
        for blk in range(NBLK):
            be_pack[:, s * NBLK + blk] = bias_even[128 * blk : 128 * blk + 128]
            bo_pack[:, s * NBLK + blk] = bias_odd[128 * blk : 128 * blk + 128]

    return dict(
        mzt=mzt_pack.astype(BF16NP),
        be=be_pack.astype(np.float32),
        bo=bo_pack.astype(np.float32),
        w1t=W1.T.astype(np.float32),
    )


def _build_kernel():
    """Build the Bass module (same program for every core)."""
    nc = bacc.Bacc("TRN2", target_bir_lowering=False, debug=False)

    y1t_d = nc.dram_tensor("y1t", [D, BS], F32, kind="ExternalInput").ap()
    w1t_d = nc.dram_tensor("w1t", [D, H], F32, kind="ExternalInput").ap()
    mzt_d = nc.dram_tensor("mzt", [128, NBLK * NBLK * 128], BF16, kind="ExternalInput").ap()
    be_d = nc.dram_tensor("be", [128, NSTEP * NBLK], F32, kind="ExternalInput").ap()
    bo_d = nc.dram_tensor("bo", [128, NSTEP * NBLK], F32, kind="ExternalInput").ap()

    ae_out_d = nc.dram_tensor("ae_out", [128, ACOLS], BF16, kind="ExternalOutput").ap()
    ao_out_d = nc.dram_tensor("ao_out", [128, ACOLS], BF16, kind="ExternalOutput").ap()

    kappa = INVL ** (-np.arange(NSTEP + 1, dtype=np.float64))

    with tile.TileContext(nc) as tc, ExitStack() as ctx:
        consts = ctx.enter_context(tc.tile_pool(name="consts", bufs=1))
        psum = ctx.enter_context(tc.tile_pool(name="psum", bufs=1, space="PSUM"))
        gpool = ctx.enter_context(tc.tile_pool(name="gps", bufs=2, space="PSUM"))
        wpool = ctx.enter_context(tc.tile_pool(name="wstate", bufs=3))
        ppool = ctx.enter_context(tc.tile_pool(name="ptmp", bufs=2))

        # --- prime the tanh activation table early (dep-free) ---
        warm = consts.tile([1, 8], F32, tag="warm")
        nc.vector.memset(warm[:], 0.0)
        nc.scalar.activation(warm[:], warm[:], mybir.ActivationFunctionType.Tanh)

        # --- load constants ---
        y1t = consts.tile([D, BS], F32, tag="y1t")
        nc.sync.dma_start(y1t[:], y1t_d)
        w1t = consts.tile([D, H], F32, tag="w1t")
        nc.sync.dma_start(w1t[:], w1t_d)
        mzt = consts.tile([128, NBLK * NBLK * 128], BF16, tag="mzt")
        nc.sync.dma_start(mzt[:], mzt_d)
        be = consts.tile([128, NSTEP * NBLK], F32, tag="be")
        nc.sync.dma_start(be[:], be_d)
        bo = consts.tile([128, NSTEP * NBLK], F32, tag="bo")
        nc.sync.dma_start(bo[:], bo_d)

        # --- activation stream buffers (written by ACT, read by PE + DMA) ---
        abuf_e = consts.tile([128, ACOLS], BF16, tag="abuf_e")
        abuf_o = consts.tile([128, ACOLS], BF16, tag="abuf_o")

        def mzt_blk(k, j):
            base = (k * NBLK + j) * 128
            return mzt[:, base : base + 128]

        # --- persistent PSUM accumulator: Z state ---
        z_ps = psum.tile([128, FREE], F32, tag="z")

        # --- init: Z_0 = W1 @ y1 ---
        # start=True only on the very first write (clears has_written for the
        # bank); stop=True on the last init MM closes the sim's race-check
        # group so later mid-accumulation PSUM reads are accepted. The
        # per-step accumulating MMs use skip_group_check (HW semantics:
        # start=False accumulates regardless; stop is sim-only).
        for j in range(NBLK):
            nc.tensor.matmul(
                z_ps[:, j * BS : (j + 1) * BS],
                w1t[:, 128 * j : 128 * j + 128],
                y1t[:],
                start=(j == 0),
                stop=(j == NBLK - 1),
            )

        # W_0 = copy of Z_0
        w_st = wpool.tile([128, FREE], F32, tag="w")
        nc.vector.tensor_copy(w_st[:], z_ps[:])

        for s in range(NSTEP):
            last = s == NSTEP - 1
            ecol = s * FREE  # column base in the stream buffers

            # --- even eval: a_even = tanh(scale * W + bias_even) ---
            a_even = abuf_e[:, ecol : ecol + FREE]
            scale_s = float(INVL**s)  # 1/kappa[s]
            for blk in range(NBLK):
                nc.scalar.activation(
                    a_even[:, blk * BS : (blk + 1) * BS],
                    w_st[:, blk * BS : (blk + 1) * BS],
                    mybir.ActivationFunctionType.Tanh,
                    bias=be[:, s * NBLK + blk : s * NBLK + blk + 1],
                    scale=scale_s,
                )

            # --- Z += Mz @ a_even (j=0 outputs first so odd ACT can start) ---
            for j in range(NBLK):
                for k in range(NBLK):
                    nc.tensor.matmul(
                        z_ps[:, j * BS : (j + 1) * BS],
                        mzt_blk(k, j),
                        a_even[:, k * BS : (k + 1) * BS],
                        start=False,
                        stop=False,
                        skip_group_check=True,
                    )

            # --- STT#1 (per block): p = c1_s * Z + W_s (DVE, overlaps odd ACT) ---
            if not last:
                p_t = ppool.tile([128, FREE], F32, tag="p")
                c1_s = float(kappa[s + 1] * (1.0 - INVL))
                for blk in range(NBLK):
                    sl = slice(blk * BS, (blk + 1) * BS)
                    nc.vector.scalar_tensor_tensor(
                        p_t[:, sl], z_ps[:, sl], c1_s, w_st[:, sl],
                        mybir.AluOpType.mult, mybir.AluOpType.add,
                    )

            # --- odd eval: a_odd = tanh(Z + bias_odd) ---
            a_odd = abuf_o[:, ecol : ecol + FREE]
            for blk in range(NBLK):
                nc.scalar.activation(
                    a_odd[:, blk * BS : (blk + 1) * BS],
                    z_ps[:, blk * BS : (blk + 1) * BS],
                    mybir.ActivationFunctionType.Tanh,
                    bias=bo[:, s * NBLK + blk : s * NBLK + blk + 1],
                    scale=1.0,
                )

            if not last:
                # --- G = Mz @ a_odd (fresh PSUM group; j=0 first) ---
                g_ps = gpool.tile([128, FREE], F32, tag="g")
                first_mm = True
                for j in range(NBLK):
                    for k in range(NBLK):
                        nc.tensor.matmul(
                            g_ps[:, j * BS : (j + 1) * BS],
                            mzt_blk(k, j),
                            a_odd[:, k * BS : (k + 1) * BS],
                            start=first_mm,
                            stop=(k == NBLK - 1 and j == NBLK - 1),
                        )
                        first_mm = False

                # --- STT#2 (per block): W_{s+1} = g2_s * G + p ---
                w_new = wpool.tile([128, FREE], F32, tag="w")
                g2_s = float(kappa[s])
                for blk in range(NBLK):
                    sl = slice(blk * BS, (blk + 1) * BS)
                    nc.vector.scalar_tensor_tensor(
                        w_new[:, sl], g_ps[:, sl], g2_s, p_t[:, sl],
                        mybir.AluOpType.mult, mybir.AluOpType.add,
                    )
                w_st = w_new

            # --- stream out completed chunks ---
            if (s + 1) % CSTEPS == 0:
                c0 = (s + 1 - CSTEPS) * FREE
                c1 = (s + 1) * FREE
                nc.sync.dma_start(ae_out_d[:, c0:c1], abuf_e[:, c0:c1])
                nc.sync.dma_start(ao_out_d[:, c0:c1], abuf_o[:, c0:c1])

    nc.compile()
    return nc


_CACHE = {}


def _get_kernel():
    if "nc" not in _CACHE:
        _CACHE["nc"] = _build_kernel()
    return _CACHE["nc"]


def kernel(y1, W1, b1, u1, W2, b2, _trace=False, _trace_kwargs=None):
    y1 = np.asarray(y1)
    in_dtype = y1.dtype
    W1_ = np.asarray(W1, dtype=np.float64)
    W2_ = np.asarray(W2, dtype=np.float64)
    b2_ = np.asarray(b2, dtype=np.float64)
    tabs = _host_tables(
        np.asarray(W1), np.asarray(b1), np.asarray(u1), np.asarray(W2), np.asarray(b2)
    )

    nc = _get_kernel()

    shared = {
        "w1t": tabs["w1t"],
        "mzt": tabs["mzt"],
        "be": tabs["be"],
        "bo": tabs["bo"],
    }
    in_maps = []
    for c in range(NCORES):
        shard = y1[c * BS : (c + 1) * BS].astype(np.float32)  # [BS, D]
        m = dict(shared)
        m["y1t"] = np.ascontiguousarray(shard.T)  # [D, BS]
        in_maps.append(m)

    kw = {}
    if _trace:
        kw["trace"] = True
        if _trace_kwargs:
            kw.update(_trace_kwargs)
    res = run_bass_kernel_spmd(nc, in_maps, core_ids=list(range(NCORES)), **kw)

    # --- exact host-side output extraction ---
    gamma, c_y, c_b = _coefficients()
    cvec = np.sum(W1_ * W2_.T, axis=1)  # diag(W1@W2)
    sum_c = float(np.sum(cvec))

    out = np.zeros((B, D + 1), dtype=np.float32)
    for c in range(NCORES):
        ae = np.asarray(res.results[c]["ae_out"]).astype(np.float64)  # [128, ACOLS]
        ao = np.asarray(res.results[c]["ao_out"]).astype(np.float64)
        # columns: s*FREE + blk*BS + b  ->  a[s][128*blk+p, b]
        ae = ae.reshape(128, NSTEP, NBLK, BS)  # [p, s, blk, b]
        ao = ao.reshape(128, NSTEP, NBLK, BS)
        ae = np.moveaxis(ae, (2, 0), (1, 2)).reshape(NSTEP, H, BS)  # [s, h, b]
        ao = np.moveaxis(ao, (2, 0), (1, 2)).reshape(NSTEP, H, BS)

        # S = sum_e gamma_e a_e  [H, BS]
        S = np.einsum("s,shb->hb", gamma[0::2], ae) + np.einsum(
            "s,shb->hb", gamma[1::2], ao
        )
        shard = y1[c * BS : (c + 1) * BS].astype(np.float64)  # [BS, D]
        y_fin = c_y * shard + (W2_ @ S).T + c_b * b2_[None, :]
        # I = h (N sum_c - sum_s c . a_even^2)
        ptr = np.einsum("h,shb->b", cvec, ae**2)
        i_fin = HSTEP * (NSTEP * sum_c - ptr)
        out[c * BS : (c + 1) * BS, :D] = y_fin.astype(np.float32)
        out[c * BS : (c + 1) * BS, D] = i_fin.astype(np.float32)

    if _trace:
        return out.astype(in_dtype, copy=False), res
    return out.astype(in_dtype, copy=False)


# revision 11
# speedup vs baseline: 1.1965x; 1.1965x over previous
"""Trainium2 Bass kernel for the CNF reversible backward solve.

Math restructuring (exact, validated in fp64 against the jax reference):

The per-step recursion
    f1 = W2 tanh(W1 y + b(t1)) + b2
    z' = z - h f1
    f0 = W2 tanh(W1 z' + b(t0)) + b2
    y' = inv_l y + (1-inv_l) z' - inv_l h f0
is tracked purely in H-space (H=256) via Z = W1 z, Y = W1 y:
    a_even = tanh(Y + beta_even)        [scalar engine, per-partition bias]
    Z     += Mz @ a_even                [PSUM-resident, Mz = -h W1 W2]
    a_odd  = tanh(Z + beta_odd)
    Y'     = inv_l Y + (1-inv_l) Z + inv_l (Mz @ a_odd)  [fused DVE ops]

The device computes ONLY this chain and streams all activations a_e out to
DRAM. The D-space outputs are exact fp64 host-side postprocessing:
    y_final = c_y y1 + sum_e gamma_e (W2 @ a_e) + c_b b2
    I_final = h (N * sum(c) - sum_s c . a_even_s^2),  c = diag(W1 W2)

Sharding: data-parallel, B=256 -> 32 samples on each of 8 cores; all
parameters/tables replicated. Output gathered and assembled on host.
"""

import numpy as np
import ml_dtypes
from contextlib import ExitStack

import concourse.bass as bass
import concourse.tile as tile
from concourse import bacc, mybir
from concourse.bass_utils import run_bass_kernel_spmd

# Problem constants (hardcoded per contract)
NCORES = 8
B, D, H = 256, 64, 256
NSTEP = 64
HSTEP = 1.0 / NSTEP
LCOUP = 0.999
INVL = 1.0 / LCOUP
BS = B // NCORES  # 32 samples per core
NBLK = H // 128  # 2 h-blocks
FREE = NBLK * BS  # 64: free size of H-space tiles, layout (blk, sample)
NEVAL = 2 * NSTEP  # 128
ACOLS = NSTEP * FREE  # 4096 columns in each activation stream buffer
DMA_CHUNKS = 8
CSTEPS = NSTEP // DMA_CHUNKS  # steps per out-DMA chunk

F32 = mybir.dt.float32
BF16 = mybir.dt.bfloat16
BF16NP = ml_dtypes.bfloat16


def _coefficients():
    """Exact fp64 scalar recursions for the output-extraction weights."""
    gamma = np.zeros(NEVAL)
    la = np.zeros(NEVAL)
    alpha_y = alpha_z = 1.0
    nu_y = nu_z = 0.0
    for s in range(NSTEP):
        la[2 * s] += -HSTEP
        nu_z += -HSTEP
        gamma *= INVL
        alpha_y *= INVL
        nu_y *= INVL
        gamma += (1.0 - INVL) * la
        alpha_y += (1.0 - INVL) * alpha_z
        nu_y += (1.0 - INVL) * nu_z
        gamma[2 * s + 1] += -INVL * HSTEP
        nu_y += -INVL * HSTEP
    return gamma, alpha_y, nu_y


def _host_tables(W1, b1, u1, W2, b2):
    """All precomputed tensors, fp64 internally."""
    W1 = W1.astype(np.float64)
    W2 = W2.astype(np.float64)
    b1 = b1.astype(np.float64)
    u1 = u1.astype(np.float64)
    b2 = b2.astype(np.float64)

    Mz = -HSTEP * (W1 @ W2)  # [H, H]
    W1b2 = W1 @ b2  # [H]

    kappa = INVL ** (-np.arange(NSTEP + 1, dtype=np.float64))
    d = np.zeros((NSTEP + 1, H))
    for s in range(NSTEP):
        d[s + 1] = d[s] - kappa[s] * HSTEP * W1b2

    # mzt_pack[p, (k*NBLK+j)*128 + m] = Mz[128*j+m, 128*k+p]
    MzT = Mz.T  # [K(h), M(h')]
    mzt_pack = np.zeros((128, NBLK * NBLK * 128))
    for k in range(NBLK):
        for j in range(NBLK):
            mzt_pack[:, (k * NBLK + j) * 128 : (k * NBLK + j + 1) * 128] = MzT[
                128 * k : 128 * k + 128, 128 * j : 128 * j + 128
            ]

    # bias tables [128, NSTEP*NBLK], col s*NBLK+blk
    be_pack = np.zeros((128, NSTEP * NBLK))
    bo_pack = np.zeros((128, NSTEP * NBLK))
    for s in range(NSTEP):
        t1 = 1.0 - s * HSTEP
        t0 = 1.0 - (s + 1) * HSTEP
        bias_even = b1 + t1 * u1 - (1.0 / kappa[s]) * d[s]
        bias_odd = b1 + t0 * u1 - (s + 1) * HSTEP * W1b2
        for blk in range(NBLK):
            be_pack[:, s * NBLK + blk] = bias_even[128 * blk : 128 * blk + 128]
            bo_pack[:, s * NBLK + blk] = bias_odd[128 * blk : 128 * blk + 128]

    return dict(
        mzt=mzt_pack.astype(BF16NP),
        be=be_pack.astype(np.float32),
        bo=bo_pack.astype(np.float32),
        w1t=W1.T.astype(np.float32),
    )


def _build_kernel():
    """Build the Bass module (same program for every core)."""
    nc = bacc.Bacc("TRN2", target_bir_lowering=False, debug=False)

    y1t_d = nc.dram_tensor("y1t", [D, BS], F32, kind="ExternalInput").ap()
    w1t_d = nc.dram_tensor("w1t", [D, H], F32, kind="ExternalInput").ap()
    mzt_d = nc.dram_tensor("mzt", [128, NBLK * NBLK * 128], BF16, kind="ExternalInput").ap()
    be_d = nc.dram_tensor("be", [128, NSTEP * NBLK], F32, kind="ExternalInput").ap()
    bo_d = nc.dram_tensor("bo", [128, NSTEP * NBLK], F32, kind="ExternalInput").ap()

    ae_out_d = nc.dram_tensor("ae_out", [128, ACOLS], BF16, kind="ExternalOutput").ap()
    ao_out_d = nc.dram_tensor("ao_out", [128, ACOLS], BF16, kind="ExternalOutput").ap()

    kappa = INVL ** (-np.arange(NSTEP + 1, dtype=np.float64))

    with tile.TileContext(nc) as tc, ExitStack() as ctx:
        consts = ctx.enter_context(tc.tile_pool(name="consts", bufs=1))
        psum = ctx.enter_context(tc.tile_pool(name="psum", bufs=1, space="PSUM"))
        gpool = ctx.enter_context(tc.tile_pool(name="gps", bufs=2, space="PSUM"))
        wpool = ctx.enter_context(tc.tile_pool(name="wstate", bufs=3))
        ppool = ctx.enter_context(tc.tile_pool(name="ptmp", bufs=2))

        # --- prime the tanh activation table early (dep-free) ---
        warm = consts.tile([1, 8], F32, tag="warm")
        nc.vector.memset(warm[:], 0.0)
        nc.scalar.activation(warm[:], warm[:], mybir.ActivationFunctionType.Tanh)

        # --- load constants ---
        y1t = consts.tile([D, BS], F32, tag="y1t")
        nc.sync.dma_start(y1t[:], y1t_d)
        w1t = consts.tile([D, H], F32, tag="w1t")
        nc.sync.dma_start(w1t[:], w1t_d)
        mzt = consts.tile([128, NBLK * NBLK * 128], BF16, tag="mzt")
        nc.sync.dma_start(mzt[:], mzt_d)
        be = consts.tile([128, NSTEP * NBLK], F32, tag="be")
        nc.sync.dma_start(be[:], be_d)
        bo = consts.tile([128, NSTEP * NBLK], F32, tag="bo")
        nc.sync.dma_start(bo[:], bo_d)

        # --- activation stream buffers (written by ACT, read by PE + DMA) ---
        abuf_e = consts.tile([128, ACOLS], BF16, tag="abuf_e")
        abuf_o = consts.tile([128, ACOLS], BF16, tag="abuf_o")

        def mzt_blk(k, j):
            base = (k * NBLK + j) * 128
            return mzt[:, base : base + 128]

        # --- persistent PSUM accumulator: Z state ---
        z_ps = psum.tile([128, FREE], F32, tag="z")

        # --- init: Z_0 = W1 @ y1 ---
        # start=True only on the very first write (clears has_written for the
        # bank); stop=True on the last init MM closes the sim's race-check
        # group so later mid-accumulation PSUM reads are accepted. The
        # per-step accumulating MMs use skip_group_check (HW semantics:
        # start=False accumulates regardless; stop is sim-only).
        for j in range(NBLK):
            nc.tensor.matmul(
                z_ps[:, j * BS : (j + 1) * BS],
                w1t[:, 128 * j : 128 * j + 128],
                y1t[:],
                start=(j == 0),
                stop=(j == NBLK - 1),
            )

        # W_0 = copy of Z_0
        w_st = wpool.tile([128, FREE], F32, tag="w")
        nc.vector.tensor_copy(w_st[:], z_ps[:])

        for s in range(NSTEP):
            last = s == NSTEP - 1
            ecol = s * FREE  # column base in the stream buffers

            # --- even eval: a_even = tanh(scale * W + bias_even) ---
            a_even = abuf_e[:, ecol : ecol + FREE]
            scale_s = float(INVL**s)  # 1/kappa[s]
            for blk in range(NBLK):
                nc.scalar.activation(
                    a_even[:, blk * BS : (blk + 1) * BS],
                    w_st[:, blk * BS : (blk + 1) * BS],
                    mybir.ActivationFunctionType.Tanh,
                    bias=be[:, s * NBLK + blk : s * NBLK + blk + 1],
                    scale=scale_s,
                )

            # --- Z += Mz @ a_even (j=0 outputs first so odd ACT can start) ---
            for j in range(NBLK):
                for k in range(NBLK):
                    nc.tensor.matmul(
                        z_ps[:, j * BS : (j + 1) * BS],
                        mzt_blk(k, j),
                        a_even[:, k * BS : (k + 1) * BS],
                        start=False,
                        stop=False,
                        skip_group_check=True,
                    )

            # --- odd eval: a_odd = tanh(Z + bias_odd) ---
            # (emitted BEFORE STT#1: the scheduler serializes same-PSUM-bank
            # accesses in program order, so STT#1 must come after the ACTs to
            # stay off the critical chain)
            a_odd = abuf_o[:, ecol : ecol + FREE]
            for blk in range(NBLK):
                nc.scalar.activation(
                    a_odd[:, blk * BS : (blk + 1) * BS],
                    z_ps[:, blk * BS : (blk + 1) * BS],
                    mybir.ActivationFunctionType.Tanh,
                    bias=bo[:, s * NBLK + blk : s * NBLK + blk + 1],
                    scale=1.0,
                )

            # --- STT#1: p = c1_s * Z + W_s (DVE, overlaps odd ACT) ---
            if not last:
                p_t = ppool.tile([128, FREE], F32, tag="p")
                c1_s = float(kappa[s + 1] * (1.0 - INVL))
                nc.vector.scalar_tensor_tensor(
                    p_t[:], z_ps[:], c1_s, w_st[:],
                    mybir.AluOpType.mult, mybir.AluOpType.add,
                )

            if not last:
                # --- G = Mz @ a_odd (fresh PSUM group; j=0 first) ---
                g_ps = gpool.tile([128, FREE], F32, tag="g")
                first_mm = True
                for j in range(NBLK):
                    for k in range(NBLK):
                        nc.tensor.matmul(
                            g_ps[:, j * BS : (j + 1) * BS],
                            mzt_blk(k, j),
                            a_odd[:, k * BS : (k + 1) * BS],
                            start=first_mm,
                            stop=(k == NBLK - 1 and j == NBLK - 1),
                        )
                        first_mm = False

                # --- STT#2: W_{s+1} = g2_s * G + p ---
                w_new = wpool.tile([128, FREE], F32, tag="w")
                g2_s = float(kappa[s])
                nc.vector.scalar_tensor_tensor(
                    w_new[:], g_ps[:], g2_s, p_t[:],
                    mybir.AluOpType.mult, mybir.AluOpType.add,
                )
                w_st = w_new

            # --- stream out completed chunks ---
            if (s + 1) % CSTEPS == 0:
                c0 = (s + 1 - CSTEPS) * FREE
                c1 = (s + 1) * FREE
                nc.sync.dma_start(ae_out_d[:, c0:c1], abuf_e[:, c0:c1])
                nc.sync.dma_start(ao_out_d[:, c0:c1], abuf_o[:, c0:c1])

    nc.compile()
    return nc


_CACHE = {}


def _get_kernel():
    if "nc" not in _CACHE:
        _CACHE["nc"] = _build_kernel()
    return _CACHE["nc"]


def kernel(y1, W1, b1, u1, W2, b2, _trace=False, _trace_kwargs=None):
    y1 = np.asarray(y1)
    in_dtype = y1.dtype
    W1_ = np.asarray(W1, dtype=np.float64)
    W2_ = np.asarray(W2, dtype=np.float64)
    b2_ = np.asarray(b2, dtype=np.float64)
    tabs = _host_tables(
        np.asarray(W1), np.asarray(b1), np.asarray(u1), np.asarray(W2), np.asarray(b2)
    )

    nc = _get_kernel()

    shared = {
        "w1t": tabs["w1t"],
        "mzt": tabs["mzt"],
        "be": tabs["be"],
        "bo": tabs["bo"],
    }
    in_maps = []
    for c in range(NCORES):
        shard = y1[c * BS : (c + 1) * BS].astype(np.float32)  # [BS, D]
        m = dict(shared)
        m["y1t"] = np.ascontiguousarray(shard.T)  # [D, BS]
        in_maps.append(m)

    kw = {}
    if _trace:
        kw["trace"] = True
        if _trace_kwargs:
            kw.update(_trace_kwargs)
    res = run_bass_kernel_spmd(nc, in_maps, core_ids=list(range(NCORES)), **kw)

    # --- exact host-side output extraction ---
    gamma, c_y, c_b = _coefficients()
    cvec = np.sum(W1_ * W2_.T, axis=1)  # diag(W1@W2)
    sum_c = float(np.sum(cvec))

    out = np.zeros((B, D + 1), dtype=np.float32)
    for c in range(NCORES):
        ae = np.asarray(res.results[c]["ae_out"]).astype(np.float64)  # [128, ACOLS]
        ao = np.asarray(res.results[c]["ao_out"]).astype(np.float64)
        # columns: s*FREE + blk*BS + b  ->  a[s][128*blk+p, b]
        ae = ae.reshape(128, NSTEP, NBLK, BS)  # [p, s, blk, b]
        ao = ao.reshape(128, NSTEP, NBLK, BS)
        ae = np.moveaxis(ae, (2, 0), (1, 2)).reshape(NSTEP, H, BS)  # [s, h, b]
        ao = np.moveaxis(ao, (2, 0), (1, 2)).reshape(NSTEP, H, BS)

        # S = sum_e gamma_e a_e  [H, BS]
        S = np.einsum("s,shb->hb", gamma[0::2], ae) + np.einsum(
            "s,shb->hb", gamma[1::2], ao
        )
        shard = y1[c * BS : (c + 1) * BS].astype(np.float64)  # [BS, D]
        y_fin = c_y * shard + (W2_ @ S).T + c_b * b2_[None, :]
        # I = h (N sum_c - sum_s c . a_even^2)
        ptr = np.einsum("h,shb->b", cvec, ae**2)
        i_fin = HSTEP * (NSTEP * sum_c - ptr)
        out[c * BS : (c + 1) * BS, :D] = y_fin.astype(np.float32)
        out[c * BS : (c + 1) * BS, D] = i_fin.astype(np.float32)

    if _trace:
        return out.astype(in_dtype, copy=False), res
    return out.astype(in_dtype, copy=False)
